# revision 25
# baseline (speedup 1.0000x reference)
"""Trainium2 Bass kernel for 3-layer GATv2 (edge features) + global pool + MLP.

v3: fp32 value path (the correctness gate needs ~1e-5 relative accuracy on
node features; 16-bit anywhere in the value/attention path fails), plus the
structural wins validated in v2:
  - batched dma_gather for per-edge source-feature fetch (one SWDGE launch
    per window x src-bucket instead of one indirect DMA per 128-edge tile);
    int16 gather indices bucketed by src row range (< SPLIT / >= SPLIT).
  - layer-1 dense transforms (x @ w_src1, x @ w_dst1) computed on the host:
    x is a replicated input, so the xs1 table and xd1 shard arrive as kernel
    inputs - no layer-1 dense phase and no layer-1 AllGather on device.
  - native parametric-relu on the Activation engine for the leaky ReLU.
  - leaner ELU finalize; batched hT write-back (both column halves in one
    Activation copy via a strided access pattern).

Distribution: edges sharded by destination node across 8 cores (dst-sorted,
window-aligned). Per-layer all-gather of the source-side transform table
(layers 2, 3); per-dst segment softmax and message aggregation fully on-core
via one-hot matmuls in PSUM.

kernel(**inputs) takes FULL inputs and returns the FULL [G, 1] output.
"""

import numpy as np

import concourse.bass as bass
import concourse.mybir as mybir
import concourse.tile as tile
from concourse import bacc, bass_utils
from concourse.masks import make_identity

F32 = mybir.dt.float32
I16 = mybir.dt.int16
AF = mybir.ActivationFunctionType
OP = mybir.AluOpType

N, E, F, ED, HID, HEADS, G = 50000, 500000, 128, 6, 64, 4, 256
HC = HEADS * HID  # 256
NEG_SLOPE = 0.2
NCORES = 8
NPC = N // NCORES      # 6250 nodes per core
WIN = 128
TILE_E = 128
NW = (NPC + WIN - 1) // WIN  # 49
GB = 4                 # tiles per compute group
SPLIT = 31250          # src-bucket boundary (both buckets < 32768 rows)

USE_PRELU = True


# ---------------------------- host-side prep --------------------------------

def _host_prep(edge_index, edge_attr, dims=None):
    n, e, ncores, npc, win, nw = dims or (N, E, NCORES, NPC, WIN, NW)
    split = min(SPLIT, ((n + 1) // 2 + 255) & ~255) if n != N else SPLIT
    src = np.asarray(edge_index[0]).astype(np.int64)
    dst = np.asarray(edge_index[1]).astype(np.int64)
    order = np.argsort(dst, kind="stable")
    s_src, s_dst = src[order], dst[order]
    s_ea = np.asarray(edge_attr, dtype=np.float32)[order]

    core = s_dst // npc
    rel = s_dst - core * npc
    wid = rel // win
    bucket = (s_src >= split).astype(np.int64)

    cnt = np.zeros((ncores, nw, 2), dtype=np.int64)
    np.add.at(cnt, (core, wid, bucket), 1)
    ntA = ((cnt[:, :, 0] + TILE_E - 1) // TILE_E).max(axis=0)
    ntB = ((cnt[:, :, 1] + TILE_E - 1) // TILE_E).max(axis=0)
    empty = (ntA + ntB) == 0
    ntA[empty] = 1
    tpw = ntA + ntB
    T = int(tpw.sum())
    tstart = np.concatenate([[0], np.cumsum(tpw)[:-1]]).astype(np.int64)

    idx16 = np.zeros((ncores, 128, 8 * T), dtype=np.int16)
    drelT = np.full((ncores, 128, T), -1.0, dtype=np.float32)
    eaT = np.zeros((ncores, ED, T * TILE_E), dtype=np.float32)

    key = (core * nw + wid) * 2 + bucket
    korder = np.argsort(key, kind="stable")
    ksorted = key[korder]
    group_first = np.concatenate([[0], np.flatnonzero(np.diff(ksorted)) + 1])
    starts = np.zeros(ncores * nw * 2, dtype=np.int64)
    starts[ksorted[group_first]] = group_first
    rank = np.empty(e, dtype=np.int64)
    rank[korder] = np.arange(e) - starts[ksorted]

    slot_base = np.where(
        bucket == 0,
        tstart[wid] * TILE_E,
        (tstart[wid] + ntA[wid]) * TILE_E,
    )
    slot = slot_base + rank

    tile_i = slot // TILE_E
    part_i = slot % TILE_E
    drelT[core, part_i, tile_i] = (rel - wid * win).astype(np.float32)
    eaT[core, :, tile_i * TILE_E + part_i] = s_ea

    # idx wrapped layout: within a gather call over tiles [t0, t0+nt), idx j
    # sits at [j % 16, 8*t0 + j // 16]
    callbase = np.where(bucket == 0, tstart[wid], tstart[wid] + ntA[wid])
    j = slot - callbase * TILE_E
    col = 8 * callbase + j // 16
    row = j % 16
    val = np.where(bucket == 0, s_src, s_src - split).astype(np.int16)
    idx16[core, row, col] = val
    # replicate the [16, cols] wrap to each 16-partition group (one copy per
    # GPSIMD core)
    idx16 = np.ascontiguousarray(np.tile(idx16[:, :16, :], (1, 8, 1)))

    return (idx16, drelT, eaT, [int(t) for t in ntA], [int(t) for t in ntB],
            T, split)


def _att_blockdiag(att):
    H, C = att.shape
    bd = np.zeros((H * C, H), dtype=np.float32)
    for h in range(H):
        bd[h * C:(h + 1) * C, h] = att[h]
    return bd


def _esel_aug(H, c_out):
    C = c_out // H
    m = np.zeros((H, c_out + H), dtype=np.float32)
    for h in range(H):
        m[h, h * C:(h + 1) * C] = 1.0
        m[h, c_out + h] = 1.0
    return m


def _khalf_pack(w):
    K, M = w.shape
    assert K % 128 == 0
    k = K // 128
    return np.concatenate([w[q * 128:(q + 1) * 128] for q in range(k)], axis=1)


# ---------------------------- kernel builder --------------------------------

class _Cfg:
    def __init__(self, n, npc, nw, ntA, ntB, ncores, g, split):
        self.n = n
        self.npc = npc
        self.nw = nw
        self.ntA = ntA
        self.ntB = ntB
        self.tpw = [a + b for a, b in zip(ntA, ntB)]
        self.T = sum(self.tpw)
        self.ncores = ncores
        self.g = g
        self.split = split
        # per layer: (k_in, c_out, H)
        self.layers = [(F, HC, HEADS), (HC, HC, HEADS), (HC, HID, 1)]


def _build(cfg: _Cfg):
    nc = bacc.Bacc(
        "TRN2", target_bir_lowering=False, debug=False,
        enable_asserts=False, num_devices=cfg.ncores,
    )

    npc, nw, T = cfg.npc, cfg.nw, cfg.T
    n_nodes, g, split = cfg.n, cfg.g, cfg.split
    tpw = cfg.tpw
    tstart = np.concatenate([[0], np.cumsum(np.asarray(tpw))[:-1]]).astype(int)

    def din(name, shape, dt=F32):
        return nc.dram_tensor(name, list(shape), dt, kind="ExternalInput").ap()

    xs1_d = din("xs_full1", [n_nodes, HC])
    xd1_d = din("xd1", [128, nw * HC])
    idx_d = din("idx16", [128, 8 * T], I16)
    drel_d = din("dstrel", [128, T])
    ea_d = din("eaT", [ED, T * TILE_E])
    batch_d = din("batchw", [128, nw])
    wcat_d = [None,
              din("wcat2", [128, 2 * 2 * HC]),
              din("wcat3", [128, 2 * 2 * HID])]
    wedge_d = [din("wedge1", [ED, HC]), din("wedge2", [ED, HC]),
               din("wedge3", [ED, HID])]
    attbd_d = [din("attbd1", [128, 2 * HEADS]), din("attbd2", [128, 2 * HEADS]),
               din("attbd3", [HID, 1])]
    esel_d = [din("esel1", [HEADS, HC + HEADS]), din("esel2", [HEADS, HC + HEADS]),
              din("esel3", [1, HID + 1])]
    bias_d = [din("bias1", [1, HC]), din("bias2", [1, HC]), din("bias3", [1, HID])]
    fc1w_d = din("fc1w", [HID, HID])
    fc1b_d = din("fc1b", [HID, 1])
    outw_d = din("outw", [HID, 1])
    outb_d = din("outb", [1, 1])
    out_d = nc.dram_tensor("out", [1, g], F32, kind="ExternalOutput").ap()

    with tile.TileContext(nc) as tc:
        res_pool_cm = tc.tile_pool(name="resident", bufs=1)
        res_pool = res_pool_cm.__enter__()

        def rtile(shape, dtype, name):
            return res_pool.tile(shape, dtype, tag=name, name=name)

        hT_sb = rtile([128, 2 * npc], F32, "hT")
        xd_sb = rtile([128, nw * HC], F32, "xd")
        h3_sb = rtile([128, nw * HID], F32, "h3")
        idx_sb = rtile([128, 8 * T], I16, "idxsb")
        drel_sb = rtile([128, T], F32, "drelsb")
        batch_sb = rtile([128, nw], F32, "batchsb")
        wcat_sb = [None] + [rtile([128, d.shape[1]], F32, f"wcat{i}")
                            for i, d in enumerate(wcat_d[1:], start=1)]
        wedge_sb = [rtile([ED, d.shape[1]], F32, f"wedge{i}")
                    for i, d in enumerate(wedge_d)]
        attbd_sb = [rtile(list(d.shape), F32, f"attbd{i}")
                    for i, d in enumerate(attbd_d)]
        esel_sb = [rtile(list(d.shape), F32, f"esel{i}")
                   for i, d in enumerate(esel_d)]
        bias_sb = [rtile([128, d.shape[1]], F32, f"biasm{i}")
                   for i, d in enumerate(bias_d)]
        fc1w_sb = rtile([HID, HID], F32, "fc1wsb")
        fc1b_sb = rtile([HID, 1], F32, "fc1bsb")
        outw_sb = rtile([HID, 1], F32, "outwsb")
        outb_sb = rtile([1, 1], F32, "outbsb")
        ident = rtile([128, 128], F32, "ident")
        iota_mat4 = rtile([128, GB * 128], F32, "iotamat4")
        giota = rtile([128, g], F32, "giota")

        nc.sync.dma_start(xd_sb[:, :], xd1_d[:, :])
        nc.gpsimd.memset(hT_sb[:, :], 0.0)
        nc.sync.dma_start(idx_sb[:, :], idx_d[:, :])
        nc.sync.dma_start(drel_sb[:, :], drel_d[:, :])
        nc.sync.dma_start(batch_sb[:, :], batch_d[:, :])
        for sb, d in zip(wcat_sb[1:] + wedge_sb + attbd_sb + esel_sb,
                         wcat_d[1:] + wedge_d + attbd_d + esel_d):
            nc.sync.dma_start(sb[:, :], d[:, :])
        for sb, d in zip([fc1w_sb, fc1b_sb, outw_sb, outb_sb],
                         [fc1w_d, fc1b_d, outw_d, outb_d]):
            nc.sync.dma_start(sb[:, :], d[:, :])
        for sb, d in zip(bias_sb, bias_d):
            nc.sync.dma_start(sb[:, :], d[0:1, :].to_broadcast([128, d.shape[1]]))

        make_identity(nc, ident[:, :])
        im_i = rtile([128, GB * 128], mybir.dt.int32, "im_i")
        gi_i = rtile([128, g], mybir.dt.int32, "gi_i")
        nc.gpsimd.iota(im_i[:, :].rearrange("p (a b) -> p a b", a=GB),
                       pattern=[[0, GB], [1, 128]], base=0, channel_multiplier=0)
        nc.gpsimd.iota(gi_i[:, :], pattern=[[1, g]], base=0, channel_multiplier=0)
        nc.vector.tensor_copy(iota_mat4[:, :], im_i[:, :])
        nc.vector.tensor_copy(giota[:, :], gi_i[:, :])

        with tc.tile_pool(name="dram", bufs=1, space="DRAM") as dpool:
            xs_shard_l = [
                None,
                dpool.tile([npc, HC], F32, name="xs_shard2"),
                dpool.tile([npc, HID], F32, name="xs_shard3"),
            ]
            xs_full_l = [
                xs1_d,
                dpool.tile([n_nodes, HC], F32, addr_space="Shared",
                           name="xs_full2"),
                dpool.tile([n_nodes, HID], F32, addr_space="Shared",
                           name="xs_full3"),
            ]
            pool_in = dpool.tile([HID, g], F32, name="pool_in")
            pool_out = dpool.tile([HID, g], F32, addr_space="Shared",
                                  name="pool_out")

            for li, (k_in, c_out, H) in enumerate(cfg.layers):
                khalves = k_in // 128
                chalves = (c_out + 127) // 128
                CA = c_out + H
                C = c_out // H
                cw0 = min(128, c_out)
                xs_full = xs_full_l[li]

                if li > 0:
                    xs_shard = xs_shard_l[li]
                    # ---------- dense phase: xd shard + xs shard ----------
                    with tc.tile_pool(name=f"dps{li}", bufs=2,
                                      space="PSUM") as psd_p, \
                         tc.tile_pool(name=f"dsb{li}", bufs=3) as dsb_p:
                        for w in range(nw):
                            nn_ = min(WIN, npc - w * WIN)
                            psd = psd_p.tile([128, 2 * c_out], F32, tag="psd")
                            for q in range(khalves):
                                lhsT = hT_sb[:, q * npc + w * WIN:
                                             q * npc + w * WIN + nn_]
                                rhs = wcat_sb[li][:, q * 2 * c_out:
                                                  (q + 1) * 2 * c_out]
                                nc.tensor.matmul(psd[:nn_, :], lhsT, rhs,
                                                 start=(q == 0),
                                                 stop=(q == khalves - 1))
                            nc.vector.tensor_copy(
                                xd_sb[:nn_, w * c_out:(w + 1) * c_out],
                                psd[:nn_, :c_out])
                            xs_stage = dsb_p.tile([128, c_out], F32,
                                                  tag="xs_stage")
                            nc.scalar.activation(xs_stage[:nn_, :c_out],
                                                 psd[:nn_, c_out:], AF.Copy)
                            nc.sync.dma_start(
                                xs_shard[w * WIN: w * WIN + nn_, :],
                                xs_stage[:nn_, :])

                    # ---------- all-gather xs ----------
                    if cfg.ncores == 1:
                        nc.sync.dma_start(xs_full[:npc, :], xs_shard[:, :])
                    else:
                        nc.gpsimd.collective_compute(
                            "AllGather", OP.bypass,
                            replica_groups=[list(range(cfg.ncores))],
                            ins=[xs_shard.opt()], outs=[xs_full.opt()],
                        )

                # ---------- edge phase ----------
                with tc.tile_pool(name=f"eg{li}", bufs=2) as g_p, \
                     tc.tile_pool(name=f"ea{li}", bufs=2) as ea_p, \
                     tc.tile_pool(name=f"oh{li}", bufs=2) as oh_p, \
                     tc.tile_pool(name=f"zt{li}", bufs=2) as zt_p, \
                     tc.tile_pool(name=f"ms{li}", bufs=3) as ms_p, \
                     tc.tile_pool(name=f"et{li}", bufs=2) as et_p, \
                     tc.tile_pool(name=f"fin{li}", bufs=1) as fin_p, \
                     tc.tile_pool(name=f"ptt{li}", bufs=2, space="PSUM") as ptt_p, \
                     tc.tile_pool(name=f"pst{li}", bufs=2, space="PSUM") as pst_p, \
                     tc.tile_pool(name=f"psA{li}", bufs=2, space="PSUM") as psA_p, \
                     tc.tile_pool(name=f"pac{li}", bufs=2, space="PSUM") as pac_p:
                    for w in range(nw):
                        nn_ = min(WIN, npc - w * WIN)
                        nA, nB = cfg.ntA[w], cfg.ntB[w]
                        ntile = nA + nB
                        t0w = int(tstart[w])
                        acc = pac_p.tile([128, CA], F32, tag="acc")
                        eaW = ea_p.tile([ED, ntile * TILE_E], F32, tag="eaW")
                        nc.sync.dma_start(
                            eaW[:, :ntile * TILE_E],
                            ea_d[:, t0w * TILE_E:(t0w + ntile) * TILE_E])
                        xs_g = g_p.tile([128, ntile, c_out], F32, tag="xs_g")
                        if nA:
                            nc.gpsimd.dma_gather(
                                out_ap=xs_g[:, 0:nA, :],
                                in_ap=xs_full[0:split, :],
                                idxs_ap=idx_sb[:, 8 * t0w: 8 * (t0w + nA)],
                                num_idxs=nA * TILE_E,
                                num_idxs_reg=nA * TILE_E,
                                elem_size=c_out,
                            )
                        if nB:
                            nc.gpsimd.dma_gather(
                                out_ap=xs_g[:, nA:ntile, :],
                                in_ap=xs_full[split:n_nodes, :],
                                idxs_ap=idx_sb[:, 8 * (t0w + nA):
                                               8 * (t0w + ntile)],
                                num_idxs=nB * TILE_E,
                                num_idxs_reg=nB * TILE_E,
                                elem_size=c_out,
                            )
                        ti = 0
                        for g0 in range(0, ntile, GB):
                            gs = min(GB, ntile - g0)
                            ew = gs * TILE_E
                            t = t0w + g0
                            S4 = oh_p.tile([128, GB * 128], F32, tag="S4")
                            nc.vector.tensor_tensor(
                                S4[:, :].rearrange(
                                    "p (a b) -> p a b", a=GB)[:, :gs, :],
                                drel_sb[:, t:t + gs].to_broadcast([128, gs, 128]),
                                iota_mat4[:, :].rearrange(
                                    "p (a b) -> p a b", a=GB)[:, :gs, :],
                                op=OP.is_equal)
                            ST4 = oh_p.tile([128, GB * 128], F32, tag="ST4")
                            for k in range(gs):
                                stp = pst_p.tile([128, 128], F32, tag="stp")
                                nc.tensor.transpose(
                                    stp[:, :], S4[:, k * 128:(k + 1) * 128],
                                    ident[:, :])
                                nc.scalar.activation(
                                    ST4[:, k * 128:(k + 1) * 128], stp[:, :],
                                    AF.Copy)
                            lg = psA_p.tile([128, 512], F32, tag="psA")
                            zT = zt_p.tile([cw0, chalves * GB * TILE_E], F32,
                                           tag="zT")
                            for q in range(chalves):
                                cw = min(128, c_out - q * 128)
                                tT = ptt_p.tile([cw0, 512], F32, tag="tT")
                                sl = tT[:cw, :ew]
                                nc.tensor.matmul(
                                    sl, wedge_sb[li][:, q * 128:q * 128 + cw],
                                    eaW[:, g0 * TILE_E: g0 * TILE_E + ew],
                                    start=True, stop=False)
                                nc.tensor.matmul(
                                    sl,
                                    xd_sb[:, w * c_out + q * 128:
                                          w * c_out + q * 128 + cw],
                                    ST4[:, :ew], start=False, stop=False)
                                for k in range(gs):
                                    nc.tensor.matmul(
                                        tT[:cw, k * TILE_E:(k + 1) * TILE_E],
                                        xs_g[:, g0 + k, q * 128:q * 128 + cw],
                                        ident[:, :], is_transpose=True,
                                        start=False, stop=(k == gs - 1))
                                zsl = zT[:cw, q * GB * TILE_E:
                                         q * GB * TILE_E + ew]
                                if USE_PRELU:
                                    nc.scalar.activation(
                                        zsl, tT[:cw, :ew], AF.Prelu,
                                        alpha=NEG_SLOPE)
                                else:
                                    abT = zt_p.tile([cw0, GB * TILE_E], F32,
                                                    tag="abT")
                                    nc.scalar.activation(
                                        abT[:cw, :ew], tT[:cw, :ew],
                                        AF.Abs, scale=(1.0 - NEG_SLOPE) / 2)
                                    nc.vector.scalar_tensor_tensor(
                                        zsl, tT[:cw, :ew],
                                        (1.0 + NEG_SLOPE) / 2, abT[:cw, :ew],
                                        op0=OP.mult, op1=OP.add)
                                nc.tensor.matmul(
                                    lg[:H, :ew],
                                    attbd_sb[li][:cw, q * H:(q + 1) * H],
                                    zsl, start=(q == 0),
                                    stop=(q == chalves - 1))
                            eT = et_p.tile([H, GB * TILE_E], F32, tag="eT")
                            nc.scalar.activation(eT[:, :ew], lg[:H, :ew], AF.Exp)
                            for k in range(gs):
                                # per-edge alpha in edge-major via a tiny PE
                                # transpose; head-block broadcast on DVE
                                alp = pst_p.tile([128, H], F32, tag="stp")
                                nc.tensor.transpose(
                                    alp[:, :H],
                                    eT[:, k * TILE_E:(k + 1) * TILE_E],
                                    ident[:H, :H])
                                msg = ms_p.tile([128, CA], F32, tag="msg")
                                nc.vector.tensor_tensor(
                                    msg[:, :c_out].rearrange(
                                        "p (h c) -> p h c", h=H),
                                    xs_g[:, g0 + k, :].rearrange(
                                        "p (h c) -> p h c", h=H),
                                    alp[:, :H].to_broadcast([128, H, C]),
                                    op=OP.mult)
                                nc.vector.tensor_copy(
                                    msg[:, c_out:], alp[:, :H])
                                nc.tensor.matmul(
                                    acc[:, :], S4[:, k * 128:(k + 1) * 128],
                                    msg[:, :], start=(ti == 0),
                                    stop=(ti == ntile - 1))
                                ti += 1
                        # ---- window finalize ----
                        dn = fin_p.tile([128, H], F32, tag="dn")
                        nc.vector.tensor_scalar_add(dn[:, :], acc[:, c_out:],
                                                    1e-16)
                        rcp = fin_p.tile([128, H], F32, tag="rcp")
                        nc.vector.reciprocal(rcp[:, :], dn[:, :])
                        t1 = fin_p.tile([128, c_out], F32, tag="t1")
                        nc.vector.tensor_tensor(
                            t1[:, :].rearrange("p (h c) -> p h c", h=H),
                            acc[:, :c_out].rearrange("p (h c) -> p h c", h=H),
                            rcp[:, :].to_broadcast([128, H, C]),
                            op=OP.mult)
                        vv = fin_p.tile([128, c_out], F32, tag="vv")
                        nc.vector.tensor_tensor(
                            vv[:, :], t1[:, :], bias_sb[li][:, :], op=OP.add)
                        # elu(v) = relu(v) + exp(v - relu(v)) - 1
                        rp = fin_p.tile([128, c_out], F32, tag="rp")
                        nc.scalar.activation(rp[:, :], vv[:, :], AF.Relu)
                        tmp = fin_p.tile([128, c_out], F32, tag="tmp")
                        nc.vector.tensor_tensor(
                            tmp[:, :], vv[:, :], rp[:, :], op=OP.subtract)
                        em = fin_p.tile([128, c_out], F32, tag="t1")
                        nc.scalar.activation(em[:, :], tmp[:, :], AF.Exp)
                        hn = fin_p.tile([128, c_out], F32, tag="vv")
                        nc.vector.scalar_tensor_tensor(
                            hn[:, :], em[:, :], -1.0, rp[:, :],
                            op0=OP.add, op1=OP.add)
                        if li < 2:
                            htp = psA_p.tile([128, 512], F32, tag="psA")
                            for q in range(chalves):
                                nc.tensor.transpose(
                                    htp[:, q * 128:(q + 1) * 128],
                                    hn[:, q * 128:(q + 1) * 128],
                                    ident[:, :])
                            # both halves in one ACT copy via strided out AP
                            hT_dst = hT_sb[:, :].rearrange(
                                "p (q n) -> p q n", q=2)[
                                :, :, w * WIN: w * WIN + nn_]
                            nc.scalar.activation(
                                hT_dst,
                                htp[:, :256].rearrange(
                                    "p (q n) -> p q n", q=2)[:, :, :nn_],
                                AF.Copy)
                        else:
                            nc.scalar.activation(
                                h3_sb[:, w * HID:(w + 1) * HID], hn[:, :],
                                AF.Copy)

            # ---------------- pooling ----------------
            with tc.tile_pool(name="poolp", bufs=2, space="PSUM") as pp_p, \
                 tc.tile_pool(name="pools", bufs=3) as ps_p:
                gps = pp_p.tile([HID, g], F32, tag="gps")
                for w in range(nw):
                    Sg = ps_p.tile([128, g], F32, tag="Sg")
                    nc.vector.tensor_tensor(
                        Sg[:, :], batch_sb[:, w:w + 1].to_broadcast([128, g]),
                        giota[:, :], op=OP.is_equal)
                    nc.tensor.matmul(gps[:, :], h3_sb[:, w * HID:(w + 1) * HID],
                                     Sg[:, :], start=(w == 0), stop=(w == nw - 1))
                gsb = ps_p.tile([HID, g], F32, tag="gsb")
                nc.vector.tensor_copy(gsb[:, :], gps[:, :])
                nc.sync.dma_start(pool_in[:, :], gsb[:, :])
                if cfg.ncores == 1:
                    nc.sync.dma_start(pool_out[:, :], pool_in[:, :])
                else:
                    nc.gpsimd.collective_compute(
                        "AllReduce", OP.add,
                        replica_groups=[list(range(cfg.ncores))],
                        ins=[pool_in.opt()], outs=[pool_out.opt()],
                    )
                pooled = ps_p.tile([HID, g], F32, tag="pooled")
                nc.sync.dma_start(pooled[:, :], pool_out[:, :])
                yps = pp_p.tile([HID, g], F32, tag="yps")
                nc.tensor.matmul(yps[:, :], fc1w_sb[:, :], pooled[:, :],
                                 start=True, stop=True)
                v1 = ps_p.tile([HID, g], F32, tag="v1")
                nc.vector.tensor_scalar_add(v1[:, :], yps[:, :], fc1b_sb[:, 0:1])
                mn1 = ps_p.tile([HID, g], F32, tag="mn1")
                nc.vector.tensor_scalar_min(mn1[:, :], v1[:, :], 0.0)
                em1 = ps_p.tile([HID, g], F32, tag="em1")
                nc.scalar.activation(em1[:, :], mn1[:, :], AF.Exp)
                rp1 = ps_p.tile([HID, g], F32, tag="rp1")
                nc.vector.tensor_scalar_max(rp1[:, :], v1[:, :], 0.0)
                y1 = ps_p.tile([HID, g], F32, tag="y1")
                nc.vector.scalar_tensor_tensor(
                    y1[:, :], em1[:, :], -1.0, rp1[:, :], op0=OP.add, op1=OP.add)
                ops_ = pp_p.tile([1, g], F32, tag="ops")
                nc.tensor.matmul(ops_[:, :], outw_sb[:, :], y1[:, :],
                                 start=True, stop=True)
                ores = ps_p.tile([1, g], F32, tag="ores")
                nc.vector.tensor_scalar_add(ores[:, :], ops_[:, :],
                                            outb_sb[0:1, 0:1])
                nc.sync.dma_start(out_d[:, :], ores[:, :])

        res_pool_cm.__exit__(None, None, None)

    nc.compile()
    return nc


# ---------------------------- public entry ----------------------------------

_CACHE = {}


def _prepare(inputs):
    idx16, drelT, eaT, ntA, ntB, T, split = _host_prep(
        inputs["edge_index"], inputs["edge_attr"])

    def f32(a):
        return np.ascontiguousarray(np.asarray(a, np.float32))

    x = f32(inputs["x"])
    batch = np.asarray(inputs["batch"]).astype(np.int64)

    # host-computed layer-1 dense transforms
    xs1 = np.ascontiguousarray(x @ f32(inputs["w_src1"]))   # [N, HC]
    xd1 = x @ f32(inputs["w_dst1"])                         # [N, HC]

    wcat2 = _khalf_pack(
        np.concatenate([f32(inputs["w_dst2"]), f32(inputs["w_src2"])], axis=1))
    wcat3 = _khalf_pack(
        np.concatenate([f32(inputs["w_dst3"]), f32(inputs["w_src3"])], axis=1))
    attbd1 = _khalf_pack(_att_blockdiag(f32(inputs["att1"])))
    attbd2 = _khalf_pack(_att_blockdiag(f32(inputs["att2"])))
    attbd3 = _att_blockdiag(f32(inputs["att3"]))

    shared = {
        "xs_full1": xs1,
        "wcat2": wcat2, "wcat3": wcat3,
        "wedge1": f32(inputs["w_edge1"]), "wedge2": f32(inputs["w_edge2"]),
        "wedge3": f32(inputs["w_edge3"]),
        "attbd1": attbd1, "attbd2": attbd2, "attbd3": attbd3,
        "esel1": _esel_aug(HEADS, HC), "esel2": _esel_aug(HEADS, HC),
        "esel3": _esel_aug(1, HID),
        "bias1": f32(inputs["b1"]).reshape(1, HC),
        "bias2": f32(inputs["b2"]).reshape(1, HC),
        "bias3": f32(inputs["b3"]).reshape(1, HID),
        "fc1w": f32(inputs["fc1_w"]), "fc1b": f32(inputs["fc1_b"]).reshape(HID, 1),
        "outw": f32(inputs["out_w"]), "outb": f32(inputs["out_b"]).reshape(1, 1),
    }

    in_maps = []
    for c in range(NCORES):
        xd1c = np.zeros((128, NW * HC), np.float32)
        bw = np.full((128, NW), -1.0, np.float32)
        bs = batch[c * NPC:(c + 1) * NPC].astype(np.float32)
        for w in range(NW):
            nn_ = min(WIN, NPC - w * WIN)
            base = c * NPC + w * WIN
            xd1c[:nn_, w * HC:(w + 1) * HC] = xd1[base: base + nn_]
            bw[:nn_, w] = bs[w * WIN: w * WIN + nn_]
        m = {"xd1": xd1c, "idx16": np.ascontiguousarray(idx16[c]),
             "dstrel": f32(drelT[c]), "eaT": f32(eaT[c]), "batchw": bw}
        m.update(shared)
        in_maps.append(m)
    return in_maps, ntA, ntB, T


LAST_RESULT = None


def kernel(**inputs) -> np.ndarray:
    global LAST_RESULT
    import os
    in_maps, ntA, ntB, T = _prepare(inputs)
    key = (T, tuple(ntA), tuple(ntB))
    if key not in _CACHE:
        cfg = _Cfg(N, NPC, NW, ntA, ntB, NCORES, G, SPLIT)
        _CACHE[key] = _build(cfg)
    nc = _CACHE[key]
    trace = os.environ.get("GAT_TRACE", "") == "1"
    res = bass_utils.run_bass_kernel_spmd(
        nc, in_maps, core_ids=list(range(NCORES)), trace=trace)
    LAST_RESULT = res
    out = res.results[0]["out"]  # [1, G]
    return np.ascontiguousarray(out.reshape(G, 1).astype(np.float32))


# revision 26
# speedup vs baseline: 1.2941x; 1.2941x over previous
"""Trainium2 Bass kernel for 3-layer GATv2 (edge features) + global pool + MLP.

v3: fp32 value path (the correctness gate needs ~1e-5 relative accuracy on
node features; 16-bit anywhere in the value/attention path fails), plus the
structural wins validated in v2:
  - batched dma_gather for per-edge source-feature fetch (one SWDGE launch
    per window x src-bucket instead of one indirect DMA per 128-edge tile);
    int16 gather indices bucketed by src row range (< SPLIT / >= SPLIT).
  - layer-1 dense transforms (x @ w_src1, x @ w_dst1) computed on the host:
    x is a replicated input, so the xs1 table and xd1 shard arrive as kernel
    inputs - no layer-1 dense phase and no layer-1 AllGather on device.
  - native parametric-relu on the Activation engine for the leaky ReLU.
  - leaner ELU finalize; batched hT write-back (both column halves in one
    Activation copy via a strided access pattern).

Distribution: edges sharded by destination node across 8 cores (dst-sorted,
window-aligned). Per-layer all-gather of the source-side transform table
(layers 2, 3); per-dst segment softmax and message aggregation fully on-core
via one-hot matmuls in PSUM.

kernel(**inputs) takes FULL inputs and returns the FULL [G, 1] output.
"""

import numpy as np

import concourse.bass as bass
import concourse.mybir as mybir
import concourse.tile as tile
from concourse import bacc, bass_utils
from concourse.masks import make_identity

F32 = mybir.dt.float32
I16 = mybir.dt.int16
AF = mybir.ActivationFunctionType
OP = mybir.AluOpType

N, E, F, ED, HID, HEADS, G = 50000, 500000, 128, 6, 64, 4, 256
HC = HEADS * HID  # 256
NEG_SLOPE = 0.2
NCORES = 8
NPC = N // NCORES      # 6250 nodes per core
WIN = 128
TILE_E = 128
NW = (NPC + WIN - 1) // WIN  # 49
GB = 4                 # tiles per compute group
SPLIT = 31250          # src-bucket boundary (both buckets < 32768 rows)

USE_PRELU = True


# ---------------------------- host-side prep --------------------------------

def _host_prep(edge_index, edge_attr, dims=None):
    n, e, ncores, npc, win, nw = dims or (N, E, NCORES, NPC, WIN, NW)
    split = min(SPLIT, ((n + 1) // 2 + 255) & ~255) if n != N else SPLIT
    src = np.asarray(edge_index[0]).astype(np.int64)
    dst = np.asarray(edge_index[1]).astype(np.int64)
    order = np.argsort(dst, kind="stable")
    s_src, s_dst = src[order], dst[order]
    s_ea = np.asarray(edge_attr, dtype=np.float32)[order]

    core = s_dst // npc
    rel = s_dst - core * npc
    wid = rel // win
    bucket = (s_src >= split).astype(np.int64)

    cnt = np.zeros((ncores, nw, 2), dtype=np.int64)
    np.add.at(cnt, (core, wid, bucket), 1)
    ntA = ((cnt[:, :, 0] + TILE_E - 1) // TILE_E).max(axis=0)
    ntB = ((cnt[:, :, 1] + TILE_E - 1) // TILE_E).max(axis=0)
    empty = (ntA + ntB) == 0
    ntA[empty] = 1
    tpw = ntA + ntB
    T = int(tpw.sum())
    tstart = np.concatenate([[0], np.cumsum(tpw)[:-1]]).astype(np.int64)

    idx16 = np.zeros((ncores, 128, 8 * T), dtype=np.int16)
    drelT = np.full((ncores, 128, T), -1.0, dtype=np.float32)
    eaT = np.zeros((ncores, ED, T * TILE_E), dtype=np.float32)

    key = (core * nw + wid) * 2 + bucket
    korder = np.argsort(key, kind="stable")
    ksorted = key[korder]
    group_first = np.concatenate([[0], np.flatnonzero(np.diff(ksorted)) + 1])
    starts = np.zeros(ncores * nw * 2, dtype=np.int64)
    starts[ksorted[group_first]] = group_first
    rank = np.empty(e, dtype=np.int64)
    rank[korder] = np.arange(e) - starts[ksorted]

    slot_base = np.where(
        bucket == 0,
        tstart[wid] * TILE_E,
        (tstart[wid] + ntA[wid]) * TILE_E,
    )
    slot = slot_base + rank

    tile_i = slot // TILE_E
    part_i = slot % TILE_E
    drelT[core, part_i, tile_i] = (rel - wid * win).astype(np.float32)
    eaT[core, :, tile_i * TILE_E + part_i] = s_ea

    # idx wrapped layout: within a gather call over tiles [t0, t0+nt), idx j
    # sits at [j % 16, 8*t0 + j // 16]
    callbase = np.where(bucket == 0, tstart[wid], tstart[wid] + ntA[wid])
    j = slot - callbase * TILE_E
    col = 8 * callbase + j // 16
    row = j % 16
    val = np.where(bucket == 0, s_src, s_src - split).astype(np.int16)
    idx16[core, row, col] = val
    # replicate the [16, cols] wrap to each 16-partition group (one copy per
    # GPSIMD core)
    idx16 = np.ascontiguousarray(np.tile(idx16[:, :16, :], (1, 8, 1)))

    return (idx16, drelT, eaT, [int(t) for t in ntA], [int(t) for t in ntB],
            T, split)


def _att_blockdiag(att):
    H, C = att.shape
    bd = np.zeros((H * C, H), dtype=np.float32)
    for h in range(H):
        bd[h * C:(h + 1) * C, h] = att[h]
    return bd


def _esel_aug(H, c_out):
    C = c_out // H
    m = np.zeros((H, c_out + H), dtype=np.float32)
    for h in range(H):
        m[h, h * C:(h + 1) * C] = 1.0
        m[h, c_out + h] = 1.0
    return m


def _khalf_pack(w):
    K, M = w.shape
    assert K % 128 == 0
    k = K // 128
    return np.concatenate([w[q * 128:(q + 1) * 128] for q in range(k)], axis=1)


# ---------------------------- kernel builder --------------------------------

class _Cfg:
    def __init__(self, n, npc, nw, ntA, ntB, ncores, g, split):
        self.n = n
        self.npc = npc
        self.nw = nw
        self.ntA = ntA
        self.ntB = ntB
        self.tpw = [a + b for a, b in zip(ntA, ntB)]
        self.T = sum(self.tpw)
        self.ncores = ncores
        self.g = g
        self.split = split
        # per layer: (k_in, c_out, H)
        self.layers = [(F, HC, HEADS), (HC, HC, HEADS), (HC, HID, 1)]


def _build(cfg: _Cfg):
    nc = bacc.Bacc(
        "TRN2", target_bir_lowering=False, debug=False,
        enable_asserts=False, num_devices=cfg.ncores,
    )

    npc, nw, T = cfg.npc, cfg.nw, cfg.T
    n_nodes, g, split = cfg.n, cfg.g, cfg.split
    tpw = cfg.tpw
    tstart = np.concatenate([[0], np.cumsum(np.asarray(tpw))[:-1]]).astype(int)

    def din(name, shape, dt=F32):
        return nc.dram_tensor(name, list(shape), dt, kind="ExternalInput").ap()

    xs1_d = din("xs_full1", [n_nodes, HC])
    xd1_d = din("xd1", [128, nw * HC])
    idx_d = din("idx16", [128, 8 * T], I16)
    drel_d = din("dstrel", [128, T])
    ea_d = din("eaT", [ED, T * TILE_E])
    batch_d = din("batchw", [128, nw])
    wcat_d = [None,
              din("wcat2", [128, 2 * 2 * HC]),
              din("wcat3", [128, 2 * 2 * HID])]
    wedge_d = [din("wedge1", [ED, HC]), din("wedge2", [ED, HC]),
               din("wedge3", [ED, HID])]
    attbd_d = [din("attbd1", [128, 2 * HEADS]), din("attbd2", [128, 2 * HEADS]),
               din("attbd3", [HID, 1])]
    esel_d = [din("esel1", [HEADS, HC + HEADS]), din("esel2", [HEADS, HC + HEADS]),
              din("esel3", [1, HID + 1])]
    bias_d = [din("bias1", [1, HC]), din("bias2", [1, HC]), din("bias3", [1, HID])]
    fc1w_d = din("fc1w", [HID, HID])
    fc1b_d = din("fc1b", [HID, 1])
    outw_d = din("outw", [HID, 1])
    outb_d = din("outb", [1, 1])
    out_d = nc.dram_tensor("out", [1, g], F32, kind="ExternalOutput").ap()

    with tile.TileContext(nc) as tc:
        res_pool_cm = tc.tile_pool(name="resident", bufs=1)
        res_pool = res_pool_cm.__enter__()

        def rtile(shape, dtype, name):
            return res_pool.tile(shape, dtype, tag=name, name=name)

        hT_sb = rtile([128, 2 * npc], F32, "hT")
        xd_sb = rtile([128, nw * HC], F32, "xd")
        h3_sb = rtile([128, nw * HID], F32, "h3")
        idx_sb = rtile([128, 8 * T], I16, "idxsb")
        drel_sb = rtile([128, T], F32, "drelsb")
        batch_sb = rtile([128, nw], F32, "batchsb")
        wcat_sb = [None] + [rtile([128, d.shape[1]], F32, f"wcat{i}")
                            for i, d in enumerate(wcat_d[1:], start=1)]
        wedge_sb = [rtile([ED, d.shape[1]], F32, f"wedge{i}")
                    for i, d in enumerate(wedge_d)]
        attbd_sb = [rtile(list(d.shape), F32, f"attbd{i}")
                    for i, d in enumerate(attbd_d)]
        esel_sb = [rtile(list(d.shape), F32, f"esel{i}")
                   for i, d in enumerate(esel_d)]
        bias_sb = [rtile([128, d.shape[1]], F32, f"biasm{i}")
                   for i, d in enumerate(bias_d)]
        fc1w_sb = rtile([HID, HID], F32, "fc1wsb")
        fc1b_sb = rtile([HID, 1], F32, "fc1bsb")
        outw_sb = rtile([HID, 1], F32, "outwsb")
        outb_sb = rtile([1, 1], F32, "outbsb")
        ident = rtile([128, 128], F32, "ident")
        iota_mat4 = rtile([128, GB * 128], F32, "iotamat4")
        giota = rtile([128, g], F32, "giota")

        nc.sync.dma_start(xd_sb[:, :], xd1_d[:, :])
        nc.gpsimd.memset(hT_sb[:, :], 0.0)
        nc.sync.dma_start(idx_sb[:, :], idx_d[:, :])
        nc.sync.dma_start(drel_sb[:, :], drel_d[:, :])
        nc.sync.dma_start(batch_sb[:, :], batch_d[:, :])
        for sb, d in zip(wcat_sb[1:] + wedge_sb + attbd_sb + esel_sb,
                         wcat_d[1:] + wedge_d + attbd_d + esel_d):
            nc.sync.dma_start(sb[:, :], d[:, :])
        for sb, d in zip([fc1w_sb, fc1b_sb, outw_sb, outb_sb],
                         [fc1w_d, fc1b_d, outw_d, outb_d]):
            nc.sync.dma_start(sb[:, :], d[:, :])
        for sb, d in zip(bias_sb, bias_d):
            nc.sync.dma_start(sb[:, :], d[0:1, :].to_broadcast([128, d.shape[1]]))

        make_identity(nc, ident[:, :])
        im_i = rtile([128, GB * 128], mybir.dt.int32, "im_i")
        gi_i = rtile([128, g], mybir.dt.int32, "gi_i")
        nc.gpsimd.iota(im_i[:, :].rearrange("p (a b) -> p a b", a=GB),
                       pattern=[[0, GB], [1, 128]], base=0, channel_multiplier=0)
        nc.gpsimd.iota(gi_i[:, :], pattern=[[1, g]], base=0, channel_multiplier=0)
        nc.vector.tensor_copy(iota_mat4[:, :], im_i[:, :])
        nc.vector.tensor_copy(giota[:, :], gi_i[:, :])

        with tc.tile_pool(name="dram", bufs=1, space="DRAM") as dpool:
            xs_shard_l = [
                None,
                dpool.tile([npc, HC], F32, name="xs_shard2"),
                dpool.tile([npc, HID], F32, name="xs_shard3"),
            ]
            xs_full_l = [
                xs1_d,
                dpool.tile([n_nodes, HC], F32, addr_space="Shared",
                           name="xs_full2"),
                dpool.tile([n_nodes, HID], F32, addr_space="Shared",
                           name="xs_full3"),
            ]
            pool_in = dpool.tile([HID, g], F32, name="pool_in")
            pool_out = dpool.tile([HID, g], F32, addr_space="Shared",
                                  name="pool_out")

            for li, (k_in, c_out, H) in enumerate(cfg.layers):
                khalves = k_in // 128
                chalves = (c_out + 127) // 128
                CA = c_out + H
                C = c_out // H
                cw0 = min(128, c_out)
                xs_full = xs_full_l[li]

                if li > 0:
                    xs_shard = xs_shard_l[li]
                    # ---------- dense phase: xd shard + xs shard ----------
                    with tc.tile_pool(name=f"dps{li}", bufs=2,
                                      space="PSUM") as psd_p, \
                         tc.tile_pool(name=f"dsb{li}", bufs=3) as dsb_p:
                        for w in range(nw):
                            nn_ = min(WIN, npc - w * WIN)
                            psd = psd_p.tile([128, 2 * c_out], F32, tag="psd")
                            for q in range(khalves):
                                lhsT = hT_sb[:, q * npc + w * WIN:
                                             q * npc + w * WIN + nn_]
                                rhs = wcat_sb[li][:, q * 2 * c_out:
                                                  (q + 1) * 2 * c_out]
                                nc.tensor.matmul(psd[:nn_, :], lhsT, rhs,
                                                 start=(q == 0),
                                                 stop=(q == khalves - 1))
                            nc.vector.tensor_copy(
                                xd_sb[:nn_, w * c_out:(w + 1) * c_out],
                                psd[:nn_, :c_out])
                            xs_stage = dsb_p.tile([128, c_out], F32,
                                                  tag="xs_stage")
                            nc.scalar.activation(xs_stage[:nn_, :c_out],
                                                 psd[:nn_, c_out:], AF.Copy)
                            nc.sync.dma_start(
                                xs_shard[w * WIN: w * WIN + nn_, :],
                                xs_stage[:nn_, :])

                    # ---------- all-gather xs ----------
                    if cfg.ncores == 1:
                        nc.sync.dma_start(xs_full[:npc, :], xs_shard[:, :])
                    else:
                        nc.gpsimd.collective_compute(
                            "AllGather", OP.bypass,
                            replica_groups=[list(range(cfg.ncores))],
                            ins=[xs_shard.opt()], outs=[xs_full.opt()],
                        )

                # ---------- edge phase ----------
                with tc.tile_pool(name=f"eg{li}", bufs=2) as g_p, \
                     tc.tile_pool(name=f"ea{li}", bufs=2) as ea_p, \
                     tc.tile_pool(name=f"oh{li}", bufs=2) as oh_p, \
                     tc.tile_pool(name=f"zt{li}", bufs=2) as zt_p, \
                     tc.tile_pool(name=f"ms{li}", bufs=3) as ms_p, \
                     tc.tile_pool(name=f"et{li}", bufs=2) as et_p, \
                     tc.tile_pool(name=f"fin{li}", bufs=1) as fin_p, \
                     tc.tile_pool(name=f"ptt{li}", bufs=2, space="PSUM") as ptt_p, \
                     tc.tile_pool(name=f"pst{li}", bufs=2, space="PSUM") as pst_p, \
                     tc.tile_pool(name=f"psA{li}", bufs=2, space="PSUM") as psA_p, \
                     tc.tile_pool(name=f"pac{li}", bufs=2, space="PSUM") as pac_p:
                    for w in range(nw):
                        nn_ = min(WIN, npc - w * WIN)
                        nA, nB = cfg.ntA[w], cfg.ntB[w]
                        ntile = nA + nB
                        t0w = int(tstart[w])
                        acc = pac_p.tile([128, CA], F32, tag="acc")
                        eaW = ea_p.tile([ED, ntile * TILE_E], F32, tag="eaW")
                        nc.sync.dma_start(
                            eaW[:, :ntile * TILE_E],
                            ea_d[:, t0w * TILE_E:(t0w + ntile) * TILE_E])
                        xs_g = g_p.tile([128, ntile, c_out], F32, tag="xs_g")
                        if nA:
                            nc.gpsimd.dma_gather(
                                out_ap=xs_g[:, 0:nA, :],
                                in_ap=xs_full[0:split, :],
                                idxs_ap=idx_sb[:, 8 * t0w: 8 * (t0w + nA)],
                                num_idxs=nA * TILE_E,
                                num_idxs_reg=nA * TILE_E,
                                elem_size=c_out,
                            )
                        if nB:
                            nc.gpsimd.dma_gather(
                                out_ap=xs_g[:, nA:ntile, :],
                                in_ap=xs_full[split:n_nodes, :],
                                idxs_ap=idx_sb[:, 8 * (t0w + nA):
                                               8 * (t0w + ntile)],
                                num_idxs=nB * TILE_E,
                                num_idxs_reg=nB * TILE_E,
                                elem_size=c_out,
                            )
                        ti = 0
                        for g0 in range(0, ntile, GB):
                            gs = min(GB, ntile - g0)
                            ew = gs * TILE_E
                            t = t0w + g0
                            S4 = oh_p.tile([128, GB * 128], F32, tag="S4")
                            nc.vector.tensor_tensor(
                                S4[:, :].rearrange(
                                    "p (a b) -> p a b", a=GB)[:, :gs, :],
                                drel_sb[:, t:t + gs].to_broadcast([128, gs, 128]),
                                iota_mat4[:, :].rearrange(
                                    "p (a b) -> p a b", a=GB)[:, :gs, :],
                                op=OP.is_equal)
                            ST4 = oh_p.tile([128, GB * 128], F32, tag="ST4")
                            for k in range(gs):
                                stp = pst_p.tile([128, 128], F32, tag="stp")
                                nc.tensor.transpose(
                                    stp[:, :], S4[:, k * 128:(k + 1) * 128],
                                    ident[:, :])
                                nc.scalar.activation(
                                    ST4[:, k * 128:(k + 1) * 128], stp[:, :],
                                    AF.Copy)
                            lg = psA_p.tile([128, 512], F32, tag="psA")
                            zT = zt_p.tile([cw0, chalves * GB * TILE_E], F32,
                                           tag="zT")
                            for q in range(chalves):
                                cw = min(128, c_out - q * 128)
                                tT = ptt_p.tile([cw0, 512], F32, tag="tT")
                                sl = tT[:cw, :ew]
                                nc.tensor.matmul(
                                    sl, wedge_sb[li][:, q * 128:q * 128 + cw],
                                    eaW[:, g0 * TILE_E: g0 * TILE_E + ew],
                                    start=True, stop=False)
                                nc.tensor.matmul(
                                    sl,
                                    xd_sb[:, w * c_out + q * 128:
                                          w * c_out + q * 128 + cw],
                                    ST4[:, :ew], start=False, stop=False)
                                for k in range(gs):
                                    nc.tensor.matmul(
                                        tT[:cw, k * TILE_E:(k + 1) * TILE_E],
                                        xs_g[:, g0 + k, q * 128:q * 128 + cw],
                                        ident[:, :], is_transpose=True,
                                        start=False, stop=(k == gs - 1))
                                zsl = zT[:cw, q * GB * TILE_E:
                                         q * GB * TILE_E + ew]
                                if USE_PRELU:
                                    nc.scalar.activation(
                                        zsl, tT[:cw, :ew], AF.Prelu,
                                        alpha=NEG_SLOPE)
                                else:
                                    abT = zt_p.tile([cw0, GB * TILE_E], F32,
                                                    tag="abT")
                                    nc.scalar.activation(
                                        abT[:cw, :ew], tT[:cw, :ew],
                                        AF.Abs, scale=(1.0 - NEG_SLOPE) / 2)
                                    nc.vector.scalar_tensor_tensor(
                                        zsl, tT[:cw, :ew],
                                        (1.0 + NEG_SLOPE) / 2, abT[:cw, :ew],
                                        op0=OP.mult, op1=OP.add)
                                nc.tensor.matmul(
                                    lg[:H, :ew],
                                    attbd_sb[li][:cw, q * H:(q + 1) * H],
                                    zsl, start=(q == 0),
                                    stop=(q == chalves - 1))
                            eT = et_p.tile([H, GB * TILE_E], F32, tag="eT")
                            nc.scalar.activation(eT[:, :ew], lg[:H, :ew], AF.Exp)
                            for k in range(gs):
                                er = psA_p.tile([128, 512], F32, tag="psA")
                                nc.tensor.matmul(
                                    er[:, :CA],
                                    eT[:, k * TILE_E:(k + 1) * TILE_E],
                                    esel_sb[li][:, :], start=True, stop=True)
                                msg = ms_p.tile([128, CA], F32, tag="msg")
                                nc.vector.tensor_tensor(
                                    msg[:, :c_out], xs_g[:, g0 + k, :],
                                    er[:, :c_out], op=OP.mult)
                                nc.vector.tensor_copy(
                                    msg[:, c_out:], er[:, c_out:CA])
                                nc.tensor.matmul(
                                    acc[:, :], S4[:, k * 128:(k + 1) * 128],
                                    msg[:, :], start=(ti == 0),
                                    stop=(ti == ntile - 1))
                                ti += 1
                        # ---- window finalize ----
                        dn = fin_p.tile([128, H], F32, tag="dn")
                        nc.vector.tensor_scalar_add(dn[:, :], acc[:, c_out:],
                                                    1e-16)
                        rcp = fin_p.tile([128, H], F32, tag="rcp")
                        nc.vector.reciprocal(rcp[:, :], dn[:, :])
                        t1 = fin_p.tile([128, c_out], F32, tag="t1")
                        nc.vector.tensor_tensor(
                            t1[:, :].rearrange("p (h c) -> p h c", h=H),
                            acc[:, :c_out].rearrange("p (h c) -> p h c", h=H),
                            rcp[:, :].to_broadcast([128, H, C]),
                            op=OP.mult)
                        vv = fin_p.tile([128, c_out], F32, tag="vv")
                        nc.vector.tensor_tensor(
                            vv[:, :], t1[:, :], bias_sb[li][:, :], op=OP.add)
                        # elu(v) = relu(v) + exp(v - relu(v)) - 1
                        rp = fin_p.tile([128, c_out], F32, tag="rp")
                        nc.scalar.activation(rp[:, :], vv[:, :], AF.Relu)
                        tmp = fin_p.tile([128, c_out], F32, tag="tmp")
                        nc.vector.tensor_tensor(
                            tmp[:, :], vv[:, :], rp[:, :], op=OP.subtract)
                        em = fin_p.tile([128, c_out], F32, tag="t1")
                        nc.scalar.activation(em[:, :], tmp[:, :], AF.Exp)
                        hn = fin_p.tile([128, c_out], F32, tag="vv")
                        nc.vector.scalar_tensor_tensor(
                            hn[:, :], em[:, :], -1.0, rp[:, :],
                            op0=OP.add, op1=OP.add)
                        if li < 2:
                            htp = psA_p.tile([128, 512], F32, tag="psA")
                            for q in range(chalves):
                                nc.tensor.transpose(
                                    htp[:, q * 128:(q + 1) * 128],
                                    hn[:, q * 128:(q + 1) * 128],
                                    ident[:, :])
                            # both halves in one ACT copy via strided out AP
                            hT_dst = hT_sb[:, :].rearrange(
                                "p (q n) -> p q n", q=2)[
                                :, :, w * WIN: w * WIN + nn_]
                            nc.scalar.activation(
                                hT_dst,
                                htp[:, :256].rearrange(
                                    "p (q n) -> p q n", q=2)[:, :, :nn_],
                                AF.Copy)
                        else:
                            nc.scalar.activation(
                                h3_sb[:, w * HID:(w + 1) * HID], hn[:, :],
                                AF.Copy)

            # ---------------- pooling ----------------
            with tc.tile_pool(name="poolp", bufs=2, space="PSUM") as pp_p, \
                 tc.tile_pool(name="pools", bufs=3) as ps_p:
                gps = pp_p.tile([HID, g], F32, tag="gps")
                for w in range(nw):
                    Sg = ps_p.tile([128, g], F32, tag="Sg")
                    nc.vector.tensor_tensor(
                        Sg[:, :], batch_sb[:, w:w + 1].to_broadcast([128, g]),
                        giota[:, :], op=OP.is_equal)
                    nc.tensor.matmul(gps[:, :], h3_sb[:, w * HID:(w + 1) * HID],
                                     Sg[:, :], start=(w == 0), stop=(w == nw - 1))
                gsb = ps_p.tile([HID, g], F32, tag="gsb")
                nc.vector.tensor_copy(gsb[:, :], gps[:, :])
                nc.sync.dma_start(pool_in[:, :], gsb[:, :])
                if cfg.ncores == 1:
                    nc.sync.dma_start(pool_out[:, :], pool_in[:, :])
                else:
                    nc.gpsimd.collective_compute(
                        "AllReduce", OP.add,
                        replica_groups=[list(range(cfg.ncores))],
                        ins=[pool_in.opt()], outs=[pool_out.opt()],
                    )
                pooled = ps_p.tile([HID, g], F32, tag="pooled")
                nc.sync.dma_start(pooled[:, :], pool_out[:, :])
                yps = pp_p.tile([HID, g], F32, tag="yps")
                nc.tensor.matmul(yps[:, :], fc1w_sb[:, :], pooled[:, :],
                                 start=True, stop=True)
                v1 = ps_p.tile([HID, g], F32, tag="v1")
                nc.vector.tensor_scalar_add(v1[:, :], yps[:, :], fc1b_sb[:, 0:1])
                mn1 = ps_p.tile([HID, g], F32, tag="mn1")
                nc.vector.tensor_scalar_min(mn1[:, :], v1[:, :], 0.0)
                em1 = ps_p.tile([HID, g], F32, tag="em1")
                nc.scalar.activation(em1[:, :], mn1[:, :], AF.Exp)
                rp1 = ps_p.tile([HID, g], F32, tag="rp1")
                nc.vector.tensor_scalar_max(rp1[:, :], v1[:, :], 0.0)
                y1 = ps_p.tile([HID, g], F32, tag="y1")
                nc.vector.scalar_tensor_tensor(
                    y1[:, :], em1[:, :], -1.0, rp1[:, :], op0=OP.add, op1=OP.add)
                ops_ = pp_p.tile([1, g], F32, tag="ops")
                nc.tensor.matmul(ops_[:, :], outw_sb[:, :], y1[:, :],
                                 start=True, stop=True)
                ores = ps_p.tile([1, g], F32, tag="ores")
                nc.vector.tensor_scalar_add(ores[:, :], ops_[:, :],
                                            outb_sb[0:1, 0:1])
                nc.sync.dma_start(out_d[:, :], ores[:, :])

        res_pool_cm.__exit__(None, None, None)

    nc.compile()
    return nc


# ---------------------------- public entry ----------------------------------

_CACHE = {}


def _prepare(inputs):
    idx16, drelT, eaT, ntA, ntB, T, split = _host_prep(
        inputs["edge_index"], inputs["edge_attr"])

    def f32(a):
        return np.ascontiguousarray(np.asarray(a, np.float32))

    x = f32(inputs["x"])
    batch = np.asarray(inputs["batch"]).astype(np.int64)

    # host-computed layer-1 dense transforms
    xs1 = np.ascontiguousarray(x @ f32(inputs["w_src1"]))   # [N, HC]
    xd1 = x @ f32(inputs["w_dst1"])                         # [N, HC]

    wcat2 = _khalf_pack(
        np.concatenate([f32(inputs["w_dst2"]), f32(inputs["w_src2"])], axis=1))
    wcat3 = _khalf_pack(
        np.concatenate([f32(inputs["w_dst3"]), f32(inputs["w_src3"])], axis=1))
    attbd1 = _khalf_pack(_att_blockdiag(f32(inputs["att1"])))
    attbd2 = _khalf_pack(_att_blockdiag(f32(inputs["att2"])))
    attbd3 = _att_blockdiag(f32(inputs["att3"]))

    shared = {
        "xs_full1": xs1,
        "wcat2": wcat2, "wcat3": wcat3,
        "wedge1": f32(inputs["w_edge1"]), "wedge2": f32(inputs["w_edge2"]),
        "wedge3": f32(inputs["w_edge3"]),
        "attbd1": attbd1, "attbd2": attbd2, "attbd3": attbd3,
        "esel1": _esel_aug(HEADS, HC), "esel2": _esel_aug(HEADS, HC),
        "esel3": _esel_aug(1, HID),
        "bias1": f32(inputs["b1"]).reshape(1, HC),
        "bias2": f32(inputs["b2"]).reshape(1, HC),
        "bias3": f32(inputs["b3"]).reshape(1, HID),
        "fc1w": f32(inputs["fc1_w"]), "fc1b": f32(inputs["fc1_b"]).reshape(HID, 1),
        "outw": f32(inputs["out_w"]), "outb": f32(inputs["out_b"]).reshape(1, 1),
    }

    in_maps = []
    for c in range(NCORES):
        xd1c = np.zeros((128, NW * HC), np.float32)
        bw = np.full((128, NW), -1.0, np.float32)
        bs = batch[c * NPC:(c + 1) * NPC].astype(np.float32)
        for w in range(NW):
            nn_ = min(WIN, NPC - w * WIN)
            base = c * NPC + w * WIN
            xd1c[:nn_, w * HC:(w + 1) * HC] = xd1[base: base + nn_]
            bw[:nn_, w] = bs[w * WIN: w * WIN + nn_]
        m = {"xd1": xd1c, "idx16": np.ascontiguousarray(idx16[c]),
             "dstrel": f32(drelT[c]), "eaT": f32(eaT[c]), "batchw": bw}
        m.update(shared)
        in_maps.append(m)
    return in_maps, ntA, ntB, T


LAST_RESULT = None


def kernel(**inputs) -> np.ndarray:
    global LAST_RESULT
    import os
    in_maps, ntA, ntB, T = _prepare(inputs)
    key = (T, tuple(ntA), tuple(ntB))
    if key not in _CACHE:
        cfg = _Cfg(N, NPC, NW, ntA, ntB, NCORES, G, SPLIT)
        _CACHE[key] = _build(cfg)
    nc = _CACHE[key]
    trace = os.environ.get("GAT_TRACE", "") == "1"
    res = bass_utils.run_bass_kernel_spmd(
        nc, in_maps, core_ids=list(range(NCORES)), trace=trace)
    LAST_RESULT = res
    out = res.results[0]["out"]  # [1, G]
    return np.ascontiguousarray(out.reshape(G, 1).astype(np.float32))
